# revision 30
# baseline (speedup 1.0000x reference)
"""Paged causal GQA attention on 8 TRN2 NeuronCores.

Problem: query [8192, 32, 128] f32 (8 seqs x 1024 tokens), paged KV cache
[32 blocks, 256, 8, 128] f32, block_tables [8, 4] int32, causal attention
with GQA (32 q-heads, 8 kv-heads, n_rep=4), scale = 1/sqrt(128).

Sharding: one sequence per core; the paged-cache gather (block_tables) is
done host-side while slicing per-core inputs, so each core runs a dense
causal attention over its own 1024-token sequence. No collectives.

The exp is split across two engines so neither is a bottleneck:
  - ScalarE (table Exp, 25 of 36 causal 128x128 blocks, 4 activations).
  - DVE (11 blocks: the 8 diagonal blocks + 3 "lone" blocks, via a
    Schraudolph exp: one scalar_tensor_tensor computing
    int16(x*(128*log2 e) + B) whose bit pattern IS bf16(e^x); the causal
    masks ride free in the in1 bias operand, with masked lanes driven to
    int16 saturation = -0.0 in bf16). Both engines write one shared bf16
    P^T tile, so PV chains are identical to a plain bf16 kernel.
Five score passes per head (A-E, 8-12 blocks each) rotate over PSUM slots
X [128,1024], Y [128,1024], Z [128,512] so every slot has two well-spaced
uses per head; QK matmuls for the narrow D/E passes are emitted j-merged
so one K_j LDWEIGHTS feeds both. PV accumulates in 3 bank-sized tiles
(387/387/258 cols = chains 0-2, 3-5, 6-7) with a ones column per V block
giving the softmax denominator; normalization is one reciprocal + one
broadcast tensor_mul per tile on DVE, one store DMA per head. Q loads are
prefetched one head ahead, K/V three heads ahead. Software pipeline: head
h-1's PV/normalize slices are interleaved between head h's score passes.

The steady-state cadence (~4.1us/head) is the ScalarE loop: ACT-A/B/C
serialized (3x1114ns) + sem post + the next head's A-QK refilling slot Y.
All of g1's PV chains sit between passes B and C so C's QK (which reuses
pass A's PSUM slot) never beats ACT-A's release; the D-section chains pad
the C->A' gap. The warmup burst is 8x512 cold matmuls (~3.4us busy at
1.2 GHz) started off a GpSimd memset so the HAM clock gate lifts to
2.4 GHz exactly when the first real QK issues; a shorter burst leaves the
first ~6us of real matmuls at half clock. The last head stores per PV
group (3 small DMAs) so the final output drain is ~86KB, not 256KB.

Do not re-split activations per bank (ACT fixed cost ~270ns + ScalarE
FIFO make release later) and do not route any slot release through DVE
(a C-exp half on DVE measured +54us via sem-indirection serialization).
"""

import os
import sys

for _p in ("/opt/trn_rl_repo", "/root/.axon_site/_ro/trn_rl_repo"):
    if os.path.isdir(_p) and _p not in sys.path:
        sys.path.insert(0, _p)

import numpy as np
import ml_dtypes

BF16 = ml_dtypes.bfloat16

NUM_HEADS = 32
HEAD_DIM = 128
NUM_KV_HEADS = 8
N_REP = NUM_HEADS // NUM_KV_HEADS
SCALE = 0.08838834764831845
NUM_SEQS = 8
SEQ_LEN = 1024
NT = SEQ_LEN // 128
N_CORES = 8

A16 = 128.0 / np.log(2.0)        # Schraudolph slope for int16/bf16 bits
B16 = 127.0 * 128 - 0.06 * 128 + 1.667  # offset, recentred for RNE

# Block (j, i) = k-tile j, q-tile i (i >= j). DVE blocks: all diagonals
# (i, i) plus lones (0,1), (2,3), (4,5); the rest on ScalarE.
DVE_BLOCKS = {(i, i) for i in range(8)} | {(0, 1), (2, 3), (4, 5)}

# Pass table: (name, slot_parity, [(j, i0, n, col), ...]) where col is the
# column of block (j, i0) inside the pass's psum slot, n = #blocks with
# consecutive i. Emission is pass-major; each (j,...) entry is one matmul.
PASSES = [
    ("A", [(0, 2, 6, 0), (4, 6, 2, 768)]),
    ("B", [(1, 2, 6, 0), (5, 6, 2, 768)]),
    ("C", [(2, 4, 4, 0), (3, 4, 4, 512)]),
    ("D", [(6, 7, 1, 0), (0, 1, 1, 128), (2, 3, 1, 256), (4, 5, 1, 384)]),
    ("E", [(j, j, 1, 128 * j) for j in range(8)]),
]
PASS_BASE = {"A": 0, "B": 1024, "C": 2048, "D": 3072, "E": 3584}
PASS_W = {"A": 1024, "B": 1024, "C": 1024, "D": 512, "E": 1024}
# scalar activation ranges + dve stt ranges per pass (cols in slot).
# DVE is the binding engine (norms + E stt); D's exp goes fully to
# ScalarE, which has slack, keeping only the mask-needing diagonals (E)
# on DVE.
SCALAR_RANGE = {"A": ((0, 1024),), "B": ((0, 1024),), "C": ((0, 1024),),
                "D": ((0, 128),)}
DVE_RANGE = {"D": ((128, 512),), "E": ((0, 1024),)}

_PCOL = {}
for name, groups in PASSES:
    for (j, i0, n, col) in groups:
        for k in range(n):
            _PCOL[(j, i0 + k)] = PASS_BASE[name] + col + 128 * k


def _build_nc():
    import concourse.bacc as bacc
    import concourse.tile as tile
    import concourse.mybir as mybir

    f32 = mybir.dt.float32
    bf16 = mybir.dt.bfloat16
    i16 = mybir.dt.int16
    Exp = mybir.ActivationFunctionType.Exp
    Alu = mybir.AluOpType

    nc = bacc.Bacc("TRN2", target_bir_lowering=False, debug=False,
                   num_devices=N_CORES)

    qT = nc.dram_tensor("qT", [NUM_HEADS, HEAD_DIM, SEQ_LEN], bf16,
                        kind="ExternalInput").ap()
    kT = nc.dram_tensor("kT", [NUM_KV_HEADS, HEAD_DIM, SEQ_LEN], bf16,
                        kind="ExternalInput").ap()
    # v16: per kv head, [128, 8*129] with a ones column per 129-block
    v16 = nc.dram_tensor("v16", [NUM_KV_HEADS, 128, NT * 129], bf16,
                         kind="ExternalInput").ap()
    # bmask: [128, 128] f32, B16 where q >= k else -1e9 (per diag block)
    bmask = nc.dram_tensor("bmask", [128, 128], f32,
                           kind="ExternalInput").ap()
    out = nc.dram_tensor("out", [SEQ_LEN, NUM_HEADS, HEAD_DIM], bf16,
                         kind="ExternalOutput").ap()

    with tile.TileContext(nc) as tc:
        with (
            tc.tile_pool(name="qpool", bufs=6) as qpool,
            tc.tile_pool(name="kpool", bufs=NUM_KV_HEADS) as kpool,
            tc.tile_pool(name="vpool", bufs=NUM_KV_HEADS) as vpool,
            tc.tile_pool(name="cpool", bufs=1) as cpool,
            tc.tile_pool(name="ppool", bufs=3) as ppool,
            tc.tile_pool(name="opool", bufs=4) as opool,
            tc.tile_pool(name="rpool", bufs=4) as rpool,
            tc.tile_pool(name="scpool", bufs=2, space="PSUM") as scpool,
            tc.tile_pool(name="scdpool", bufs=1, space="PSUM") as scdpool,
            tc.tile_pool(name="pvpool", bufs=1, space="PSUM") as pvpool,
        ):
            bm = cpool.tile([128, 128], f32, tag="bm")
            bcol = cpool.tile([128, 1], f32, tag="bcol")
            nc.vector.memset(bcol[:, :], float(B16))

            # warm up PE p-state while the first DMAs land: HAM needs
            # ~3.4us of sustained matmul busy to lift the 1.2->2.4 GHz
            # clock gate, so burst ~8x512 cold matmuls (~3.4us at 1.2).
            # memset on GpSimd: it is idle in the preamble, so the burst
            # starts ~1us earlier than a DVE memset would allow.
            wu = cpool.tile([128, 512], bf16, tag="wu")
            nc.gpsimd.memset(wu[:, :], 0.0)
            sc_wu = scpool.tile([128, 1024], f32, tag="sc")
            for _ in range(8):
                nc.tensor.matmul(sc_wu[:, 0:512], lhsT=wu[:, 0:128],
                                 rhs=wu[:, 0:512], start=True, stop=True,
                                 skip_group_check=True)

            kts = [None] * NUM_KV_HEADS
            vts = [None] * NUM_KV_HEADS
            qts = [None] * NUM_HEADS

            def load_k(kvh):
                kt_t = kpool.tile([128, SEQ_LEN], bf16, tag="kt")
                nc.sync.dma_start(out=kt_t[:, :], in_=kT[kvh])
                kts[kvh] = kt_t

            def load_v(kvh):
                vt = vpool.tile([128, NT * 129], bf16, tag="vt")
                nc.sync.dma_start(out=vt[:, :], in_=v16[kvh])
                vts[kvh] = vt

            def load_q(h, split=False):
                qt = qpool.tile([128, SEQ_LEN], bf16, tag="qt")
                if split:
                    nc.sync.dma_start(out=qt[:, 0:512], in_=qT[h][:, 0:512])
                    nc.sync.dma_start(out=qt[:, 512:1024],
                                      in_=qT[h][:, 512:1024])
                else:
                    nc.sync.dma_start(out=qt[:, :], in_=qT[h])
                qts[h] = qt
                return qt

            def emit_qk(qt, kt_t, sc, groups, seen_banks):
                for (j, i0, n, col) in groups:
                    # split at 512-col (psum bank) boundaries
                    off = 0
                    while off < 128 * n:
                        w0 = min(512 - (col + off) % 512, 128 * n - off)
                        b = (col + off) // 512
                        start = b not in seen_banks
                        seen_banks.add(b)
                        nc.tensor.matmul(
                            sc[:, col + off:col + off + w0],
                            lhsT=kt_t[:, 128 * j:128 * (j + 1)],
                            rhs=qt[:, 128 * i0 + off:128 * i0 + off + w0],
                            start=start, stop=True,
                            skip_group_check=True,
                        )
                        off += w0

            def emit_pass(h, qt, ph, name, groups, sc=None):
                kvh = h // N_REP
                kt_t = kts[kvh]
                if sc is None:
                    if name == "D":
                        sc = scdpool.tile([128, 512], f32, tag="scd")
                    else:
                        sc = scpool.tile([128, 1024], f32, tag="sc")
                    emit_qk(qt, kt_t, sc, groups, set())
                base = PASS_BASE[name]
                if name in SCALAR_RANGE:
                    for (c0, c1) in SCALAR_RANGE[name]:
                        nc.scalar.activation(
                            ph[:, base + c0:base + c1], sc[:, c0:c1], Exp,
                            scale=SCALE)
                for (c0, c1) in DVE_RANGE.get(name, ()):
                    phi = ph[:, :].bitcast(i16)
                    if name == "E":
                        in1 = bm[:, :].rearrange(
                            "p (o c) -> p o c", o=1).broadcast_to(
                            [128, (c1 - c0) // 128, 128])
                        out_ap = phi[:, base + c0:base + c1].rearrange(
                            "p (o c) -> p o c", c=128)
                        in_ap = sc[:, c0:c1].rearrange(
                            "p (o c) -> p o c", c=128)
                    else:
                        in1 = bcol[:, 0:1].broadcast_to([128, c1 - c0])
                        out_ap = phi[:, base + c0:base + c1]
                        in_ap = sc[:, c0:c1]
                    nc.vector.scalar_tensor_tensor(
                        out_ap, in_ap, float(SCALE * A16), in1,
                        op0=Alu.mult, op1=Alu.add)

            # pv groups: (first chain, #chains); each fits one psum bank
            PV_GROUPS = [(0, 3), (3, 3), (6, 2)]

            def emit_pv_chains(h, ph, g, lo, hi):
                """PV chains lo..hi-1 (chain index within group g)."""
                vt = vts[h // N_REP]
                pv = pv_tiles[g]
                c0, _ = PV_GROUPS[g]
                for t in range(lo, hi):
                    i = c0 + t
                    for j in range(i + 1):
                        c = _PCOL[(j, i)]
                        nc.tensor.matmul(
                            pv[:, 129 * t:129 * (t + 1)],
                            lhsT=ph[:, c:c + 128],
                            rhs=vt[:, 129 * j:129 * (j + 1)],
                            start=(j == 0), stop=(j == i),
                            skip_group_check=True,
                        )

            osb_head = [None]

            def emit_pv_norm(h, g):
                c0, n = PV_GROUPS[g]
                pv = pv_tiles[g]
                pv3 = pv[:, :].rearrange("p (t c) -> p t c", c=129)
                r = rpool.tile([128, n], f32, tag="r")
                nc.vector.reciprocal(r[:, :], pv3[:, :, 128])
                if g == 0:
                    osb = opool.tile([128, 1024], bf16, tag="osb")
                    osb_head[0] = osb
                else:
                    osb = osb_head[0]
                osb3 = osb[:, 128 * c0:128 * (c0 + n)].rearrange(
                    "p (t d) -> p t d", d=128)
                r3 = r[:, :].rearrange("p (t c) -> p t c", c=1).broadcast_to(
                    [128, n, 128])
                nc.vector.tensor_tensor(osb3, pv3[:, :, 0:128], r3,
                                        op=mybir.AluOpType.mult)
                if h == NUM_HEADS - 1:
                    # last head: store per group so the final transfer is
                    # small and overlaps the remaining PV/norm work
                    dst = out[:, h, :].rearrange("(t q) d -> q t d", t=8)
                    src = osb[:, 128 * c0:128 * (c0 + n)].rearrange(
                        "p (t d) -> p t d", d=128)
                    nc.sync.dma_start(out=dst[:, c0:c0 + n, :], in_=src)
                elif g == 2:
                    # one store per head covering all 8 q-tiles
                    dst = out[:, h, :].rearrange("(t q) d -> q t d", t=8)
                    src = osb[:, :].rearrange("p (t d) -> p t d", d=128)
                    nc.sync.dma_start(out=dst, in_=src)

            pv_tiles = [None, None, None]

            prev = None
            for h in range(NUM_HEADS):
                kvh = h // N_REP
                if h == 0:
                    kt_t = kpool.tile([128, SEQ_LEN], bf16, tag="kt")
                    kts[0] = kt_t
                    qt = qpool.tile([128, SEQ_LEN], bf16, tag="qt")
                    qts[0] = qt
                    # qT descriptor first: the first real matmul is gated by
                    # this 256KB transfer (the kT half still lands before
                    # its LDWEIGHTS needs it even when issued second)
                    nc.sync.dma_start(out=qt[:, 0:1024], in_=qT[0])
                    nc.sync.dma_start(out=kt_t[:, 0:512], in_=kT[0][:, 0:512])
                    nc.sync.dma_start(out=kt_t[:, 512:1024],
                                      in_=kT[0][:, 512:1024])
                    nc.sync.dma_start(out=bm[:, :], in_=bmask)
                    load_v(0)
                    load_q(1, split=True)
                else:
                    qt = qts[h]
                if h + 1 < NUM_HEADS:
                    load_q(h + 1, split=h + 1 < 4)
                if h % N_REP == 1 and kvh + 1 < NUM_KV_HEADS:
                    load_k(kvh + 1)
                    load_v(kvh + 1)

                ph = ppool.tile([128, 4608], bf16, tag="ph")
                # pass-major emission with prev-head PV interleaved:
                # chains per slice keep PE fed between score passes
                for pi, (name, groups) in enumerate(PASSES):
                    if name == "D":
                        # merged D+E QK emission: one K_j ldweights feeds
                        # both passes' narrow matmuls
                        scD = scdpool.tile([128, 512], f32, tag="scd")
                        scE = scpool.tile([128, 1024], f32, tag="sc")
                        dmap = {j: (i0, col) for (j, i0, n, col) in groups}
                        sbD, sbE = set(), set()
                        for j in range(8):
                            if j in dmap:
                                i0, col = dmap[j]
                                emit_qk(qt, kts[kvh], scD,
                                        [(j, i0, 1, col)], sbD)
                            emit_qk(qt, kts[kvh], scE,
                                    [(j, j, 1, 128 * j)], sbE)
                        if prev is not None:
                            emit_pv_chains(prev[0], prev[1], 2, 0, 1)
                        emit_pass(h, qt, ph, "D", groups, sc=scD)
                        emit_pass(h, qt, ph, "E", PASSES[4][1], sc=scE)
                        if prev is not None:
                            # norm g1 emitted after the stt pair so the
                            # schraudolph results are ready sooner for the
                            # next head's PV weight loads
                            emit_pv_norm(prev[0], 1)
                            emit_pv_chains(prev[0], prev[1], 2, 1, 2)
                            emit_pv_norm(prev[0], 2)
                        break
                    emit_pass(h, qt, ph, name, groups)
                    if prev is not None:
                        hp, php = prev
                        if pi == 0:
                            emit_pv_chains(hp, php, 0, 0, 3)
                            emit_pv_norm(hp, 0)
                        elif pi == 1:
                            # all of g1 here: pads the A->C slot-reuse gap
                            # (C-QK stalls on ACT-A otherwise); the C->D gap
                            # has surplus padding from the D-section chains
                            emit_pv_chains(hp, php, 1, 0, 3)
                # allocate pv tiles for this head after prev's are done
                pv_a = pvpool.tile([128, 387], f32, tag="pva")
                pv_b = pvpool.tile([128, 387], f32, tag="pvb")
                pv_c = pvpool.tile([128, 258], f32, tag="pvc")
                pv_tiles[0] = pv_a
                pv_tiles[1] = pv_b
                pv_tiles[2] = pv_c
                prev = (h, ph)
            # drain last head
            for g in range(3):
                emit_pv_chains(prev[0], prev[1], g, 0, PV_GROUPS[g][1])
                emit_pv_norm(prev[0], g)

    nc.compile()
    return nc


_NC_CACHE = {}


def _get_nc():
    if "nc" not in _NC_CACHE:
        _NC_CACHE["nc"] = _build_nc()
    return _NC_CACHE["nc"]


def make_in_maps(query, k_cache, v_cache, block_tables):
    query = np.asarray(query, dtype=np.float32)
    k_cache = np.asarray(k_cache, dtype=np.float32)
    v_cache = np.asarray(v_cache, dtype=np.float32)
    block_tables = np.asarray(block_tables)

    # bmask[k, q] (within a 128 diag block): B16 where q >= k else -1e9
    keep = np.arange(128)[None, :] >= np.arange(128)[:, None]
    bmask = np.where(keep, np.float32(B16), np.float32(-1e9))

    in_maps = []
    for i in range(N_CORES):
        q_i = query[SEQ_LEN * i:SEQ_LEN * (i + 1)]
        qT_i = np.ascontiguousarray(q_i.transpose(1, 2, 0)).astype(BF16)
        blocks = block_tables[i]
        k_i = k_cache[blocks].reshape(SEQ_LEN, NUM_KV_HEADS, HEAD_DIM)
        v_i = v_cache[blocks].reshape(SEQ_LEN, NUM_KV_HEADS, HEAD_DIM)
        kT_i = np.ascontiguousarray(k_i.transpose(1, 2, 0)).astype(BF16)
        # v16: [kv, 128, 8*129], block j cols 129j:129j+128 = V rows, col
        # 129j+128 = ones
        vv = v_i.transpose(1, 0, 2).reshape(NUM_KV_HEADS, NT, 128, HEAD_DIM)
        v16 = np.ones((NUM_KV_HEADS, 128, NT, 129), np.float32)
        v16[:, :, :, 0:128] = vv.transpose(0, 2, 1, 3)
        v16 = np.ascontiguousarray(
            v16.reshape(NUM_KV_HEADS, 128, NT * 129)).astype(BF16)
        in_maps.append({
            "qT": qT_i, "kT": kT_i, "v16": v16, "bmask": bmask,
        })
    return in_maps


def kernel(query, k_cache, v_cache, block_tables):
    from concourse.bass_utils import run_bass_kernel_spmd

    in_maps = make_in_maps(query, k_cache, v_cache, block_tables)
    nc = _get_nc()
    res = run_bass_kernel_spmd(nc, in_maps, list(range(N_CORES)))
    outs = [np.asarray(res.results[i]["out"]).astype(np.float32)
            for i in range(N_CORES)]
    return np.concatenate(outs, axis=0)



# revision 31
# speedup vs baseline: 1.0144x; 1.0144x over previous
"""Paged causal GQA attention on 8 TRN2 NeuronCores.

Problem: query [8192, 32, 128] f32 (8 seqs x 1024 tokens), paged KV cache
[32 blocks, 256, 8, 128] f32, block_tables [8, 4] int32, causal attention
with GQA (32 q-heads, 8 kv-heads, n_rep=4), scale = 1/sqrt(128).

Sharding: one sequence per core; the paged-cache gather (block_tables) is
done host-side while slicing per-core inputs, so each core runs a dense
causal attention over its own 1024-token sequence. No collectives.

The exp is split across two engines so neither is a bottleneck:
  - ScalarE (table Exp, 25 of 36 causal 128x128 blocks, 4 activations).
  - DVE (11 blocks: the 8 diagonal blocks + 3 "lone" blocks, via a
    Schraudolph exp: one scalar_tensor_tensor computing
    int16(x*(128*log2 e) + B) whose bit pattern IS bf16(e^x); the causal
    masks ride free in the in1 bias operand, with masked lanes driven to
    int16 saturation = -0.0 in bf16). Both engines write one shared bf16
    P^T tile, so PV chains are identical to a plain bf16 kernel.
Five score passes per head (A-E, 8-12 blocks each) rotate over PSUM slots
X [128,1024], Y [128,1024], Z [128,512] so every slot has two well-spaced
uses per head; QK matmuls for the narrow D/E passes are emitted j-merged
so one K_j LDWEIGHTS feeds both. PV accumulates in 3 bank-sized tiles
(387/387/258 cols = chains 0-2, 3-5, 6-7) with a ones column per V block
giving the softmax denominator; normalization is one reciprocal + one
broadcast tensor_mul per tile on DVE, one store DMA per head. Q loads are
prefetched one head ahead, K/V three heads ahead. Software pipeline: head
h-1's PV/normalize slices are interleaved between head h's score passes.

The steady-state cadence (~4.1us/head) is the ScalarE loop: ACT-A/B/C
serialized (3x1114ns) + sem post + the next head's A-QK refilling slot Y.
All of g1's PV chains sit between passes B and C so C's QK (which reuses
pass A's PSUM slot) never beats ACT-A's release; the D-section chains pad
the C->A' gap. The warmup burst is 8x512 cold matmuls (~3.4us busy at
1.2 GHz) started off a GpSimd memset so the HAM clock gate lifts to
2.4 GHz exactly when the first real QK issues; a shorter burst leaves the
first ~6us of real matmuls at half clock. The last head stores per PV
group (3 small DMAs) so the final output drain is ~86KB, not 256KB.

Do not re-split activations per bank (ACT fixed cost ~270ns + ScalarE
FIFO make release later) and do not route any slot release through DVE
(a C-exp half on DVE measured +54us via sem-indirection serialization).
"""

import os
import sys

for _p in ("/opt/trn_rl_repo", "/root/.axon_site/_ro/trn_rl_repo"):
    if os.path.isdir(_p) and _p not in sys.path:
        sys.path.insert(0, _p)

import numpy as np
import ml_dtypes

BF16 = ml_dtypes.bfloat16

NUM_HEADS = 32
HEAD_DIM = 128
NUM_KV_HEADS = 8
N_REP = NUM_HEADS // NUM_KV_HEADS
SCALE = 0.08838834764831845
NUM_SEQS = 8
SEQ_LEN = 1024
NT = SEQ_LEN // 128
N_CORES = 8

A16 = 128.0 / np.log(2.0)        # Schraudolph slope for int16/bf16 bits
B16 = 127.0 * 128 - 0.06 * 128 + 1.667  # offset, recentred for RNE

# Block (j, i) = k-tile j, q-tile i (i >= j). DVE blocks: all diagonals
# (i, i) plus lones (0,1), (2,3), (4,5); the rest on ScalarE.
DVE_BLOCKS = {(i, i) for i in range(8)} | {(0, 1), (2, 3), (4, 5)}

# Pass table: (name, slot_parity, [(j, i0, n, col), ...]) where col is the
# column of block (j, i0) inside the pass's psum slot, n = #blocks with
# consecutive i. Emission is pass-major; each (j,...) entry is one matmul.
PASSES = [
    ("A", [(0, 2, 6, 0), (4, 6, 2, 768)]),
    ("B", [(1, 2, 6, 0), (5, 6, 2, 768)]),
    ("C", [(2, 4, 4, 0), (3, 4, 4, 512)]),
    ("D", [(6, 7, 1, 0), (0, 1, 1, 128), (2, 3, 1, 256), (4, 5, 1, 384)]),
    ("E", [(j, j, 1, 128 * j) for j in range(8)]),
]
PASS_BASE = {"A": 0, "B": 1024, "C": 2048, "D": 3072, "E": 3584}
PASS_W = {"A": 1024, "B": 1024, "C": 1024, "D": 512, "E": 1024}
# scalar activation ranges + dve stt ranges per pass (cols in slot).
# DVE is the binding engine (norms + E stt); D's exp goes fully to
# ScalarE, which has slack, keeping only the mask-needing diagonals (E)
# on DVE.
SCALAR_RANGE = {"A": ((0, 1024),), "B": ((0, 1024),), "C": ((0, 1024),),
                "D": ((0, 128),)}
DVE_RANGE = {"D": ((128, 512),), "E": ((0, 1024),)}

_PCOL = {}
for name, groups in PASSES:
    for (j, i0, n, col) in groups:
        for k in range(n):
            _PCOL[(j, i0 + k)] = PASS_BASE[name] + col + 128 * k


def _build_nc():
    import concourse.bacc as bacc
    import concourse.tile as tile
    import concourse.mybir as mybir

    f32 = mybir.dt.float32
    bf16 = mybir.dt.bfloat16
    i16 = mybir.dt.int16
    Exp = mybir.ActivationFunctionType.Exp
    Alu = mybir.AluOpType

    nc = bacc.Bacc("TRN2", target_bir_lowering=False, debug=False,
                   num_devices=N_CORES)

    qT = nc.dram_tensor("qT", [NUM_HEADS, HEAD_DIM, SEQ_LEN], bf16,
                        kind="ExternalInput").ap()
    kT = nc.dram_tensor("kT", [NUM_KV_HEADS, HEAD_DIM, SEQ_LEN], bf16,
                        kind="ExternalInput").ap()
    # v16: per kv head, [128, 8*129] with a ones column per 129-block
    v16 = nc.dram_tensor("v16", [NUM_KV_HEADS, 128, NT * 129], bf16,
                         kind="ExternalInput").ap()
    # bmask: [128, 128] f32, B16 where q >= k else -1e9 (per diag block)
    bmask = nc.dram_tensor("bmask", [128, 128], f32,
                           kind="ExternalInput").ap()
    out = nc.dram_tensor("out", [SEQ_LEN, NUM_HEADS, HEAD_DIM], bf16,
                         kind="ExternalOutput").ap()

    with tile.TileContext(nc) as tc:
        with (
            tc.tile_pool(name="qpool", bufs=6) as qpool,
            tc.tile_pool(name="kpool", bufs=NUM_KV_HEADS) as kpool,
            tc.tile_pool(name="vpool", bufs=NUM_KV_HEADS) as vpool,
            tc.tile_pool(name="cpool", bufs=1) as cpool,
            tc.tile_pool(name="ppool", bufs=3) as ppool,
            tc.tile_pool(name="opool", bufs=4) as opool,
            tc.tile_pool(name="rpool", bufs=4) as rpool,
            tc.tile_pool(name="scpool", bufs=2, space="PSUM") as scpool,
            tc.tile_pool(name="scdpool", bufs=1, space="PSUM") as scdpool,
            tc.tile_pool(name="pvpool", bufs=1, space="PSUM") as pvpool,
        ):
            bm = cpool.tile([128, 128], f32, tag="bm")
            bcol = cpool.tile([128, 1], f32, tag="bcol")
            nc.vector.memset(bcol[:, :], float(B16))

            # warm up PE p-state while the first DMAs land: HAM needs
            # ~3.4us of sustained matmul busy to lift the 1.2->2.4 GHz
            # clock gate, so burst ~8x512 cold matmuls (~3.4us at 1.2).
            # memset on GpSimd: it is idle in the preamble, so the burst
            # starts ~1us earlier than a DVE memset would allow.
            wu = cpool.tile([128, 512], bf16, tag="wu")
            nc.gpsimd.memset(wu[:, :], 0.0)
            sc_wu = scpool.tile([128, 1024], f32, tag="sc")
            for _ in range(8):
                nc.tensor.matmul(sc_wu[:, 0:512], lhsT=wu[:, 0:128],
                                 rhs=wu[:, 0:512], start=True, stop=True,
                                 skip_group_check=True)

            kts = [None] * NUM_KV_HEADS
            vts = [None] * NUM_KV_HEADS
            qts = [None] * NUM_HEADS

            def load_k(kvh):
                kt_t = kpool.tile([128, SEQ_LEN], bf16, tag="kt")
                nc.sync.dma_start(out=kt_t[:, :], in_=kT[kvh])
                kts[kvh] = kt_t

            def load_v(kvh):
                vt = vpool.tile([128, NT * 129], bf16, tag="vt")
                nc.sync.dma_start(out=vt[:, :], in_=v16[kvh])
                vts[kvh] = vt

            def load_q(h, split=False):
                qt = qpool.tile([128, SEQ_LEN], bf16, tag="qt")
                if split:
                    nc.sync.dma_start(out=qt[:, 0:512], in_=qT[h][:, 0:512])
                    nc.sync.dma_start(out=qt[:, 512:1024],
                                      in_=qT[h][:, 512:1024])
                else:
                    nc.sync.dma_start(out=qt[:, :], in_=qT[h])
                qts[h] = qt
                return qt

            def emit_qk(qt, kt_t, sc, groups, seen_banks):
                for (j, i0, n, col) in groups:
                    # split at 512-col (psum bank) boundaries
                    off = 0
                    while off < 128 * n:
                        w0 = min(512 - (col + off) % 512, 128 * n - off)
                        b = (col + off) // 512
                        start = b not in seen_banks
                        seen_banks.add(b)
                        nc.tensor.matmul(
                            sc[:, col + off:col + off + w0],
                            lhsT=kt_t[:, 128 * j:128 * (j + 1)],
                            rhs=qt[:, 128 * i0 + off:128 * i0 + off + w0],
                            start=start, stop=True,
                            skip_group_check=True,
                        )
                        off += w0

            def emit_pass(h, qt, ph, name, groups, sc=None):
                kvh = h // N_REP
                kt_t = kts[kvh]
                if sc is None:
                    if name == "D":
                        sc = scdpool.tile([128, 512], f32, tag="scd")
                    else:
                        sc = scpool.tile([128, 1024], f32, tag="sc")
                    emit_qk(qt, kt_t, sc, groups, set())
                base = PASS_BASE[name]
                if name in SCALAR_RANGE:
                    for (c0, c1) in SCALAR_RANGE[name]:
                        nc.scalar.activation(
                            ph[:, base + c0:base + c1], sc[:, c0:c1], Exp,
                            scale=SCALE)
                for (c0, c1) in DVE_RANGE.get(name, ()):
                    phi = ph[:, :].bitcast(i16)
                    if name == "E":
                        in1 = bm[:, :].rearrange(
                            "p (o c) -> p o c", o=1).broadcast_to(
                            [128, (c1 - c0) // 128, 128])
                        out_ap = phi[:, base + c0:base + c1].rearrange(
                            "p (o c) -> p o c", c=128)
                        in_ap = sc[:, c0:c1].rearrange(
                            "p (o c) -> p o c", c=128)
                    else:
                        in1 = bcol[:, 0:1].broadcast_to([128, c1 - c0])
                        out_ap = phi[:, base + c0:base + c1]
                        in_ap = sc[:, c0:c1]
                    nc.vector.scalar_tensor_tensor(
                        out_ap, in_ap, float(SCALE * A16), in1,
                        op0=Alu.mult, op1=Alu.add)

            # pv groups: (first chain, #chains); each fits one psum bank
            PV_GROUPS = [(0, 3), (3, 3), (6, 2)]

            def emit_pv_chains(h, ph, g, lo, hi):
                """PV chains lo..hi-1 (chain index within group g)."""
                vt = vts[h // N_REP]
                pv = pv_tiles[g]
                c0, _ = PV_GROUPS[g]
                for t in range(lo, hi):
                    i = c0 + t
                    for j in range(i + 1):
                        c = _PCOL[(j, i)]
                        nc.tensor.matmul(
                            pv[:, 129 * t:129 * (t + 1)],
                            lhsT=ph[:, c:c + 128],
                            rhs=vt[:, 129 * j:129 * (j + 1)],
                            start=(j == 0), stop=(j == i),
                            skip_group_check=True,
                        )

            osb_head = [None]

            def emit_pv_norm(h, g):
                c0, n = PV_GROUPS[g]
                pv = pv_tiles[g]
                pv3 = pv[:, :].rearrange("p (t c) -> p t c", c=129)
                r = rpool.tile([128, n], f32, tag="r")
                nc.vector.reciprocal(r[:, :], pv3[:, :, 128])
                if g == 0:
                    osb = opool.tile([128, 1024], bf16, tag="osb")
                    osb_head[0] = osb
                else:
                    osb = osb_head[0]
                osb3 = osb[:, 128 * c0:128 * (c0 + n)].rearrange(
                    "p (t d) -> p t d", d=128)
                r3 = r[:, :].rearrange("p (t c) -> p t c", c=1).broadcast_to(
                    [128, n, 128])
                nc.vector.tensor_tensor(osb3, pv3[:, :, 0:128], r3,
                                        op=mybir.AluOpType.mult)
                if h == NUM_HEADS - 1:
                    # last head: store per group so the final transfer is
                    # small and overlaps the remaining PV/norm work
                    dst = out[:, h, :].rearrange("(t q) d -> q t d", t=8)
                    src = osb[:, 128 * c0:128 * (c0 + n)].rearrange(
                        "p (t d) -> p t d", d=128)
                    nc.sync.dma_start(out=dst[:, c0:c0 + n, :], in_=src)
                elif g == 2:
                    # one store per head covering all 8 q-tiles
                    dst = out[:, h, :].rearrange("(t q) d -> q t d", t=8)
                    src = osb[:, :].rearrange("p (t d) -> p t d", d=128)
                    nc.sync.dma_start(out=dst, in_=src)

            pv_tiles = [None, None, None]

            prev = None
            for h in range(NUM_HEADS):
                kvh = h // N_REP
                if h == 0:
                    kt_t = kpool.tile([128, SEQ_LEN], bf16, tag="kt")
                    kts[0] = kt_t
                    qt = qpool.tile([128, SEQ_LEN], bf16, tag="qt")
                    qts[0] = qt
                    nc.sync.dma_start(out=kt_t[:, 0:512], in_=kT[0][:, 0:512])
                    nc.sync.dma_start(out=qt[:, 0:1024], in_=qT[0])
                    nc.sync.dma_start(out=kt_t[:, 512:1024],
                                      in_=kT[0][:, 512:1024])
                    nc.sync.dma_start(out=bm[:, :], in_=bmask)
                    load_v(0)
                    load_q(1, split=True)
                else:
                    qt = qts[h]
                if h + 1 < NUM_HEADS:
                    load_q(h + 1, split=h + 1 < 4)
                if h % N_REP == 1 and kvh + 1 < NUM_KV_HEADS:
                    load_k(kvh + 1)
                    load_v(kvh + 1)

                ph = ppool.tile([128, 4608], bf16, tag="ph")
                # pass-major emission with prev-head PV interleaved:
                # chains per slice keep PE fed between score passes
                for pi, (name, groups) in enumerate(PASSES):
                    if name == "D":
                        # merged D+E QK emission: one K_j ldweights feeds
                        # both passes' narrow matmuls
                        scD = scdpool.tile([128, 512], f32, tag="scd")
                        scE = scpool.tile([128, 1024], f32, tag="sc")
                        dmap = {j: (i0, col) for (j, i0, n, col) in groups}
                        sbD, sbE = set(), set()
                        for j in range(8):
                            if j in dmap:
                                i0, col = dmap[j]
                                emit_qk(qt, kts[kvh], scD,
                                        [(j, i0, 1, col)], sbD)
                            emit_qk(qt, kts[kvh], scE,
                                    [(j, j, 1, 128 * j)], sbE)
                        if prev is not None:
                            emit_pv_chains(prev[0], prev[1], 2, 0, 1)
                        emit_pass(h, qt, ph, "D", groups, sc=scD)
                        emit_pass(h, qt, ph, "E", PASSES[4][1], sc=scE)
                        if prev is not None:
                            # norm g1 emitted after the stt pair so the
                            # schraudolph results are ready sooner for the
                            # next head's PV weight loads
                            emit_pv_norm(prev[0], 1)
                            emit_pv_chains(prev[0], prev[1], 2, 1, 2)
                            emit_pv_norm(prev[0], 2)
                        break
                    emit_pass(h, qt, ph, name, groups)
                    if prev is not None:
                        hp, php = prev
                        if pi == 0:
                            emit_pv_chains(hp, php, 0, 0, 3)
                            emit_pv_norm(hp, 0)
                        elif pi == 1:
                            # all of g1 here: pads the A->C slot-reuse gap
                            # (C-QK stalls on ACT-A otherwise); the C->D gap
                            # has surplus padding from the D-section chains
                            emit_pv_chains(hp, php, 1, 0, 3)
                # allocate pv tiles for this head after prev's are done
                pv_a = pvpool.tile([128, 387], f32, tag="pva")
                pv_b = pvpool.tile([128, 387], f32, tag="pvb")
                pv_c = pvpool.tile([128, 258], f32, tag="pvc")
                pv_tiles[0] = pv_a
                pv_tiles[1] = pv_b
                pv_tiles[2] = pv_c
                prev = (h, ph)
            # drain last head
            for g in range(3):
                emit_pv_chains(prev[0], prev[1], g, 0, PV_GROUPS[g][1])
                emit_pv_norm(prev[0], g)

    nc.compile()
    return nc


_NC_CACHE = {}


def _get_nc():
    if "nc" not in _NC_CACHE:
        _NC_CACHE["nc"] = _build_nc()
    return _NC_CACHE["nc"]


def make_in_maps(query, k_cache, v_cache, block_tables):
    query = np.asarray(query, dtype=np.float32)
    k_cache = np.asarray(k_cache, dtype=np.float32)
    v_cache = np.asarray(v_cache, dtype=np.float32)
    block_tables = np.asarray(block_tables)

    # bmask[k, q] (within a 128 diag block): B16 where q >= k else -1e9
    keep = np.arange(128)[None, :] >= np.arange(128)[:, None]
    bmask = np.where(keep, np.float32(B16), np.float32(-1e9))

    in_maps = []
    for i in range(N_CORES):
        q_i = query[SEQ_LEN * i:SEQ_LEN * (i + 1)]
        qT_i = np.ascontiguousarray(q_i.transpose(1, 2, 0)).astype(BF16)
        blocks = block_tables[i]
        k_i = k_cache[blocks].reshape(SEQ_LEN, NUM_KV_HEADS, HEAD_DIM)
        v_i = v_cache[blocks].reshape(SEQ_LEN, NUM_KV_HEADS, HEAD_DIM)
        kT_i = np.ascontiguousarray(k_i.transpose(1, 2, 0)).astype(BF16)
        # v16: [kv, 128, 8*129], block j cols 129j:129j+128 = V rows, col
        # 129j+128 = ones
        vv = v_i.transpose(1, 0, 2).reshape(NUM_KV_HEADS, NT, 128, HEAD_DIM)
        v16 = np.ones((NUM_KV_HEADS, 128, NT, 129), np.float32)
        v16[:, :, :, 0:128] = vv.transpose(0, 2, 1, 3)
        v16 = np.ascontiguousarray(
            v16.reshape(NUM_KV_HEADS, 128, NT * 129)).astype(BF16)
        in_maps.append({
            "qT": qT_i, "kT": kT_i, "v16": v16, "bmask": bmask,
        })
    return in_maps


def kernel(query, k_cache, v_cache, block_tables):
    from concourse.bass_utils import run_bass_kernel_spmd

    in_maps = make_in_maps(query, k_cache, v_cache, block_tables)
    nc = _get_nc()
    res = run_bass_kernel_spmd(nc, in_maps, list(range(N_CORES)))
    outs = [np.asarray(res.results[i]["out"]).astype(np.float32)
            for i in range(N_CORES)]
    return np.concatenate(outs, axis=0)

